# revision 1
# baseline (speedup 1.0000x reference)
"""DeformableTransformer on 8 trn2 NeuronCores.

Sharding: data-parallel over batch (2) x token-parallel (4) => 8 cores.
All large (8192-token) matmuls run on device via a Bass/Tile matmul kernel
(lhsT = W^T resident, rhs = d-major activations). Host numpy does the
deformable trilinear sampling glue, layernorms and the tiny 100-query
decoder arithmetic between device calls.
"""

import numpy as np

import concourse.bass as bass
import concourse.mybir as mybir
import concourse.tile as tile
from concourse import bacc
from concourse.bass_utils import run_bass_kernel_spmd

# model constants (hardcoded per spec)
D3, H3, W3 = 8, 32, 32
D_MODEL, N_HEADS, N_POINTS = 256, 8, 4
D_FFN, N_LAYERS, NQ = 1024, 6, 100
EPS = 1e-5
N_TOK = D3 * H3 * W3          # 8192
N_CORES = 8
TOK_PER_CORE = N_TOK // 4     # 2048
M_PAD = 1536                  # fixed output-feature pad for the device matmul

_NC_CACHE = {}


def _build_mm_nc(K):
    """AT[K,2048] x WT[K,1536] -> OUT[1536,2048], fp32, one NeuronCore."""
    f32 = mybir.dt.float32
    nc = bacc.Bacc("TRN2", target_bir_lowering=False, debug=False,
                   num_devices=N_CORES)
    a = nc.dram_tensor("a", [K, TOK_PER_CORE], f32, kind="ExternalInput").ap()
    w = nc.dram_tensor("w", [K, M_PAD], f32, kind="ExternalInput").ap()
    o = nc.dram_tensor("o", [M_PAD, TOK_PER_CORE], f32,
                       kind="ExternalOutput").ap()
    KT = K // 128
    with tile.TileContext(nc) as tc:
        with tc.tile_pool(name="sb", bufs=1) as sb, \
             tc.tile_pool(name="ps", bufs=2, space="PSUM") as ps, \
             tc.tile_pool(name="ev", bufs=3) as ev:
            a_sb = sb.tile([128, KT, TOK_PER_CORE], f32)
            nc.sync.dma_start(a_sb[:], a.rearrange("(t p) n -> p t n", p=128))
            w_sb = sb.tile([128, KT, M_PAD], f32)
            nc.sync.dma_start(w_sb[:], w.rearrange("(t p) m -> p t m", p=128))
            for mt in range(M_PAD // 128):
                for nt in range(TOK_PER_CORE // 512):
                    pt = ps.tile([128, 512], f32)
                    for kt in range(KT):
                        nc.tensor.matmul(
                            pt[:],
                            w_sb[:, kt, 128 * mt:128 * (mt + 1)],
                            a_sb[:, kt, 512 * nt:512 * (nt + 1)],
                            start=(kt == 0), stop=(kt == KT - 1))
                    ot = ev.tile([128, 512], f32)
                    nc.any.tensor_copy(ot[:], pt[:])
                    nc.sync.dma_start(
                        o[128 * mt:128 * (mt + 1), 512 * nt:512 * (nt + 1)],
                        ot[:])
    nc.compile()
    return nc


def _mm(actT, WT):
    """actT: [2, K, 8192] fp32 d-major activations (per batch).
    WT: [K, M<=1536]. Returns [2, M, 8192] fp32 = (WT.T @ actT)."""
    K = actT.shape[1]
    M = WT.shape[1]
    if K not in _NC_CACHE:
        _NC_CACHE[K] = _build_mm_nc(K)
    nc = _NC_CACHE[K]
    wt_pad = WT
    if M < M_PAD:
        wt_pad = np.zeros((K, M_PAD), np.float32)
        wt_pad[:, :M] = WT
    in_maps = []
    for c in range(N_CORES):
        b, g = c // 4, c % 4
        in_maps.append({
            "a": np.ascontiguousarray(
                actT[b, :, TOK_PER_CORE * g: TOK_PER_CORE * (g + 1)]),
            "w": wt_pad,
        })
    res = run_bass_kernel_spmd(nc, in_maps, core_ids=list(range(N_CORES)))
    out = np.empty((2, M, N_TOK), np.float32)
    for c in range(N_CORES):
        b, g = c // 4, c % 4
        out[b, :, TOK_PER_CORE * g: TOK_PER_CORE * (g + 1)] = \
            res.results[c]["o"][:M]
    return out


# ---------------- host-side numpy pieces ----------------

def _layer_norm(x, g, b):
    m = x.mean(-1, keepdims=True)
    v = ((x - m) ** 2).mean(-1, keepdims=True)
    return (x - m) / np.sqrt(v + EPS) * g + b


def _softmax(x, axis):
    x = x - x.max(axis=axis, keepdims=True)
    e = np.exp(x)
    return e / e.sum(axis=axis, keepdims=True)


def _trilinear_sample(v, loc):
    # v: (bs, D, H, W, nh, ch); loc: (bs, Lq, nh, P, 3) xyz in [0,1]
    bs, Dd, Hh, Ww, nh, ch = v.shape
    Lq, P = loc.shape[1], loc.shape[3]
    vf = np.transpose(v.reshape(bs, Dd * Hh * Ww, nh, ch), (0, 2, 1, 3))
    sizes = np.array([Ww, Hh, Dd], loc.dtype)
    g = loc * sizes - 0.5
    g0 = np.floor(g)
    f = g - g0
    g0 = g0.astype(np.int32)
    out = np.zeros((bs, nh, Lq * P, ch), v.dtype)
    for dz in (0, 1):
        for dy in (0, 1):
            for dx in (0, 1):
                xi = np.clip(g0[..., 0] + dx, 0, Ww - 1)
                yi = np.clip(g0[..., 1] + dy, 0, Hh - 1)
                zi = np.clip(g0[..., 2] + dz, 0, Dd - 1)
                w = ((f[..., 0] if dx else 1 - f[..., 0])
                     * (f[..., 1] if dy else 1 - f[..., 1])
                     * (f[..., 2] if dz else 1 - f[..., 2]))
                idx = (zi * Hh + yi) * Ww + xi
                idxf = np.transpose(idx, (0, 2, 1, 3)).reshape(bs, nh, Lq * P)
                wf = np.transpose(w, (0, 2, 1, 3)).reshape(bs, nh, Lq * P)
                gath = np.take_along_axis(vf, idxf[..., None], axis=2)
                out = out + wf[..., None] * gath
    return np.transpose(out.reshape(bs, nh, Lq, P, ch), (0, 2, 1, 3, 4))


def _deform_attn_tail(v, off, aw, ref, out_w, out_b):
    """v: (bs, n, d) values; off: (bs, Lq, h*p*3); aw: (bs, Lq, h*p).
    Returns (bs, Lq, d) = attention output (before residual)."""
    bs, Lq = off.shape[0], off.shape[1]
    ch = D_MODEL // N_HEADS
    vv = v.reshape(bs, D3, H3, W3, N_HEADS, ch)
    offr = off.reshape(bs, Lq, N_HEADS, N_POINTS, 3)
    awr = _softmax(aw.reshape(bs, Lq, N_HEADS, N_POINTS), -1)
    sizes = np.array([W3, H3, D3], np.float32)
    loc = ref[:, :, None, None, :] + offr / sizes
    s = _trilinear_sample(vv, loc)
    out = (awr[..., None] * s).sum(3).reshape(bs, Lq, D_MODEL)
    return out @ out_w.T + out_b


def kernel(srcs, points, pos_embeds, masks, enc_params, dec_params,
           query_embed_w, ref_w, ref_b):
    srcs = np.asarray(srcs, np.float32)
    points = np.asarray(points, np.float32)
    pos_embeds = np.asarray(pos_embeds, np.float32)
    ep = {k: np.asarray(v, np.float32) for k, v in enc_params.items()}
    dp = {k: np.asarray(v, np.float32) for k, v in dec_params.items()}
    query_embed_w = np.asarray(query_embed_w, np.float32)
    ref_w = np.asarray(ref_w, np.float32)
    ref_b = np.asarray(ref_b, np.float32)

    bs = srcs.shape[0]
    mem = srcs                                   # (2, 8192, 256) token-major

    # ---- encoder: 6 layers ----
    for i in range(N_LAYERS):
        p = {k: v[i] for k, v in ep.items()}
        q = mem + pos_embeds
        memT = np.ascontiguousarray(np.transpose(mem, (0, 2, 1)))
        qT = np.ascontiguousarray(np.transpose(q, (0, 2, 1)))
        # device: v projection (from src) ; off+aw projections (from q)
        v = _mm(memT, p['val_w'].T)              # (2, 256, 8192)
        v = np.transpose(v, (0, 2, 1)) + p['val_b']
        w_offaw = np.concatenate([p['off_w'].T, p['aw_w'].T], axis=1)
        oa = _mm(qT, w_offaw)                    # (2, 96+32, 8192)
        oa = np.transpose(oa, (0, 2, 1))
        off = oa[:, :, :N_HEADS * N_POINTS * 3] + p['off_b']
        aw = oa[:, :, N_HEADS * N_POINTS * 3:] + p['aw_b']
        # host: trilinear sampling + weighted sum
        samp = _deform_attn_tail(v, off, aw, points, np.eye(D_MODEL,
                                 dtype=np.float32), np.zeros(1, np.float32))
        # device: output projection of sampled features
        sT = np.ascontiguousarray(np.transpose(samp, (0, 2, 1)))
        attn = _mm(sT, p['out_w'].T)
        attn = np.transpose(attn, (0, 2, 1)) + p['out_b']
        src1 = _layer_norm(mem + attn, p['ln1_g'], p['ln1_b'])
        # device: FFN
        s1T = np.ascontiguousarray(np.transpose(src1, (0, 2, 1)))
        h = _mm(s1T, p['lin1_w'].T)              # (2, 1024, 8192)
        h = np.maximum(h + p['lin1_b'][None, :, None], 0.0)
        h2 = _mm(h, p['lin2_w'].T)               # K=1024 kernel
        h2 = np.transpose(h2, (0, 2, 1)) + p['lin2_b']
        mem = _layer_norm(src1 + h2, p['ln2_g'], p['ln2_b'])

    # ---- decoder value projections for all 6 layers in one device call ----
    memT = np.ascontiguousarray(np.transpose(mem, (0, 2, 1)))
    wv_all = np.concatenate([dp['val_w'][i].T for i in range(N_LAYERS)],
                            axis=1)              # (256, 1536)
    v_all = _mm(memT, wv_all)                    # (2, 1536, 8192)

    # ---- decoder (host, 100 queries) ----
    qe = np.broadcast_to(query_embed_w[None, :, :D_MODEL],
                         (bs, NQ, D_MODEL)).astype(np.float32)
    tgt = np.broadcast_to(query_embed_w[None, :, D_MODEL:],
                          (bs, NQ, D_MODEL)).astype(np.float32)
    ref = 1.0 / (1.0 + np.exp(-(qe @ ref_w.T + ref_b)))
    out = tgt.copy()
    ch = D_MODEL // N_HEADS
    for i in range(N_LAYERS):
        p = {k: v[i] for k, v in dp.items()}
        qx = out + qe
        # self-attention
        wq, wk, wv = np.split(p['sa_in_w'], 3, 0)
        bq, bk, bv = np.split(p['sa_in_b'], 3, 0)
        qh = (qx @ wq.T + bq).reshape(bs, NQ, N_HEADS, ch)
        kh = (qx @ wk.T + bk).reshape(bs, NQ, N_HEADS, ch)
        vh = (out @ wv.T + bv).reshape(bs, NQ, N_HEADS, ch)
        att = np.einsum('bqhc,bkhc->bhqk', qh, kh) * (ch ** -0.5)
        att = _softmax(att, -1)
        o = np.einsum('bhqk,bkhc->bqhc', att, vh).reshape(bs, NQ, D_MODEL)
        mha_out = o @ p['sa_out_w'].T + p['sa_out_b']
        out = _layer_norm(out + mha_out, p['ln2_g'], p['ln2_b'])
        # deformable cross-attention on encoder memory
        v_i = np.transpose(v_all[:, 256 * i:256 * (i + 1), :], (0, 2, 1)) \
            + p['val_b']
        q2 = out + qe
        off = q2 @ p['off_w'].T + p['off_b']
        aw = q2 @ p['aw_w'].T + p['aw_b']
        samp = _deform_attn_tail(v_i, off, aw, ref,
                                 np.eye(D_MODEL, dtype=np.float32),
                                 np.zeros(1, np.float32))
        attn = samp @ p['out_w'].T + p['out_b']
        out = _layer_norm(out + attn, p['ln1_g'], p['ln1_b'])
        # FFN
        hh = np.maximum(out @ p['lin1_w'].T + p['lin1_b'], 0.0)
        out = _layer_norm(out + hh @ p['lin2_w'].T + p['lin2_b'],
                          p['ln3_g'], p['ln3_b'])
    return out.astype(np.float32)
